# revision 32
# baseline (speedup 1.0000x reference)
"""Multi-head causal attention with RoPE on 8 Trainium2 NeuronCores.

Sharding: data-parallel over batch (B=2) x tensor-parallel over heads
(16 heads -> 4 groups of 4). Core c handles batch c//4, heads
[(c%4)*4, (c%4)*4+4). Each core computes a partial y = attn_out @ W_o
for its head group; the host sums the 4 partials per batch (the "W_o
all-reduce").

Device kernel (per core, all matmuls bf16, fp32 PSUM accumulation):
  - x^T built on-chip via PE transposes (contraction over E needs E on
    partitions).
  - Q^T/K^T/V^T projections in "T layout" (dims on partitions, seq on
    free): out = W_chunk.T @ x^T_chunk accumulated over 8 E-chunks.
  - RoPE: the within-head pair shuffle is folded into a host-side
    permutation of W_q/W_k columns so the rotation partner sits 16
    partitions away inside the same 32-partition quadrant; on device a
    single DVE stream_shuffle + cos/sin multiply-adds apply the
    rotation. Scores are permutation-invariant since Q and K use the
    same permutation.
  - scores^T[t, q] = K^T_tile.T @ Q^T (only t-blocks <= q-block:
    causal skip), exp on ACT (scale=1/32 folded in), causal mask on
    diagonal blocks, P^T @ [V | 1] accumulated in PSUM -> out^T plus
    softmax denominators in one matmul (ones column appended to V).
  - normalize with reciprocal + gpsimd partition_broadcast, then
    y = out_norm^T.T @ W_o chunks.
"""

import os
import sys
from contextlib import ExitStack

import numpy as np

for _p in ("/opt/trn_rl_repo",):
    if os.path.isdir(_p) and _p not in sys.path:
        sys.path.insert(0, _p)

import ml_dtypes  # noqa: E402

BF16 = ml_dtypes.bfloat16

B, S, E = 2, 2048, 1024
H, DH = 16, 64
NCORES = 8
HPC = H // 4          # 4 heads per core
DC = HPC * DH         # 256 head dims per core
ATTN_SCALE = 1.0 / 32.0  # 1/sqrt(E)
ROPE_BASE = 10000.0
P = 128
NSB = S // P          # 16 sequence blocks
NEC = E // P          # 8 E chunks
MB = DC // P          # 2 partition blocks of head dims

_PROG = None


def _perm64():
    """perm[j] = original head-dim index stored at permuted position j.

    Quadrant q of the permuted layout holds RoPE pairs i in
    [16q, 16q+16): even elements (2i) at slots 0-15, odd (2i+1) at
    slots 16-31. The rotation partner is then always +-16 partitions
    away within one 32-partition quadrant (stream_shuffle range).
    """
    j = np.arange(64)
    qd, r = j // 32, j % 32
    i = 16 * qd + (r % 16)
    return 2 * i + (r >= 16)


def _cos_sin_tiles():
    pl = np.arange(P) % 64
    qd, r = pl // 32, pl % 32
    i = 16 * qd + (r % 16)
    inv = ROPE_BASE ** (-(2.0 * i) / DH)
    ang = np.arange(S)[None, :] * inv[:, None]          # (128, S)
    sgn = np.where(r < 16, -1.0, 1.0)[:, None]
    return ang, sgn


def _build_program(debug=False):
    import concourse.bacc as bacc
    import concourse.tile as tile
    from concourse import masks, mybir

    f32 = mybir.dt.float32
    bf16 = mybir.dt.bfloat16
    AF = mybir.ActivationFunctionType

    nc = bacc.Bacc("TRN2", target_bir_lowering=False, debug=False)
    xbt = nc.dram_tensor("xbt", [E, S], bf16, kind="ExternalInput").ap()
    wq = nc.dram_tensor("wq", [E, DC], bf16, kind="ExternalInput").ap()
    wk = nc.dram_tensor("wk", [E, DC], bf16, kind="ExternalInput").ap()
    wv = nc.dram_tensor("wv", [E, DC], bf16, kind="ExternalInput").ap()
    wo = nc.dram_tensor("wo", [DC, E], bf16, kind="ExternalInput").ap()
    cosr = nc.dram_tensor("cosr", [P, S], bf16, kind="ExternalInput").ap()
    sinr = nc.dram_tensor("sinr", [P, S], bf16, kind="ExternalInput").ap()
    cmask = nc.dram_tensor("cmask", [P, P], bf16, kind="ExternalInput").ap()
    y = nc.dram_tensor("y", [S, E], f32, kind="ExternalOutput").ap()
    if debug:
        dbg = {
            "dxT": nc.dram_tensor("dxT", [P, NEC, S], mybir.dt.bfloat16,
                                  kind="ExternalOutput").ap(),
            "dqcT": nc.dram_tensor("dqcT", [P, MB, S], mybir.dt.bfloat16,
                                   kind="ExternalOutput").ap(),
            "dqT": nc.dram_tensor("dqT", [P, MB, S], mybir.dt.bfloat16,
                                  kind="ExternalOutput").ap(),
            "dkT": nc.dram_tensor("dkT", [P, MB, S], mybir.dt.bfloat16,
                                  kind="ExternalOutput").ap(),
            "dvn": nc.dram_tensor("dvn", [P, NSB, HPC, 65], mybir.dt.bfloat16,
                                  kind="ExternalOutput").ap(),
            "donrm": nc.dram_tensor("donrm", [P, MB, S], mybir.dt.bfloat16,
                                    kind="ExternalOutput").ap(),
            "dacc": nc.dram_tensor("dacc", [DH, HPC, S], f32,
                                   kind="ExternalOutput").ap(),
            "dden": nc.dram_tensor("dden", [1, HPC, S], f32,
                                   kind="ExternalOutput").ap(),
        }

    with ExitStack() as ctx:
        tc = ctx.enter_context(tile.TileContext(nc))
        consts = ctx.enter_context(tc.tile_pool(name="consts", bufs=1))
        persist = ctx.enter_context(tc.tile_pool(name="persist", bufs=1))

        ident = consts.tile([P, P], bf16, tag="ident")
        masks.make_identity(nc, ident[:])

        xT = persist.tile([P, NEC, S], bf16, tag="xT")
        qcT = persist.tile([P, MB, S], bf16, tag="qcT")
        kcT = persist.tile([P, MB, S], bf16, tag="kcT")
        vT = persist.tile([P, MB, S], bf16, tag="vT")
        # qz holds RoPE'd Q^T zero-padded per head parity: slice
        # [:, mb, par, :] has head (2*mb+par)'s 64 rows live and the
        # other 64 rows zero. Scores then use the FULL 128-row K^T
        # block as lhsT (K=128): the HAM clock gate never grants full
        # clock to half-height (K=64) matmul streams, and the zero
        # rows contribute exactly 0.
        qz = persist.tile([P, MB, 2, S], bf16, tag="qz")
        kT = persist.tile([P, MB, S], bf16, tag="kT")
        vn = persist.tile([P, NSB, HPC, 65], bf16, tag="vn")
        onrm = persist.tile([P, MB, S], bf16, tag="onrm")

        # ---- Phase A: x^T, projections, RoPE, V natural ----
        with ExitStack() as actx:
            xnat = actx.enter_context(tc.tile_pool(name="xnat", bufs=3))
            tp_ps = actx.enter_context(
                tc.tile_pool(name="tp_ps", bufs=3, space="PSUM")
            )
            pr_ps = actx.enter_context(
                tc.tile_pool(name="pr_ps", bufs=2, space="PSUM")
            )
            rtmp = actx.enter_context(tc.tile_pool(name="rtmp", bufs=2))

            # x^T comes pre-transposed from the host (E, S). Two 2MB
            # halves on the two HWDGE rings, issued BEFORE the consts so
            # the projections (which accumulate over all 8 chunks) can
            # start ~7us in. Weights follow, K-projection weights first.
            half_ecs = NEC // 2
            nc.sync.dma_start(
                xT[:, 0:half_ecs, :],
                xbt[0:half_ecs * P, :].rearrange("(c p) s -> p c s", p=P),
            )
            nc.scalar.dma_start(
                xT[:, half_ecs:NEC, :],
                xbt[half_ecs * P:E, :].rearrange("(c p) s -> p c s", p=P),
            )
            wk_t = consts.tile([P, NEC, DC], bf16, tag="wk")
            nc.sync.dma_start(wk_t[:], wk.rearrange("(c p) m -> p c m", p=P))
            wq_t = consts.tile([P, NEC, DC], bf16, tag="wq")
            nc.scalar.dma_start(wq_t[:], wq.rearrange("(c p) m -> p c m", p=P))
            wv_t = consts.tile([P, NEC, DC], bf16, tag="wv")
            nc.sync.dma_start(wv_t[:], wv.rearrange("(c p) m -> p c m", p=P))
            cos_t = consts.tile([P, S], bf16, tag="cos")
            nc.scalar.dma_start(cos_t[:], cosr)
            sin_t = consts.tile([P, S], bf16, tag="sin")
            nc.sync.dma_start(sin_t[:], sinr)
            wo_t = consts.tile([P, MB, E], bf16, tag="wo")
            nc.scalar.dma_start(wo_t[:], wo.rearrange("(c p) n -> p c n", p=P))
            msk_t = consts.tile([P, P], bf16, tag="msk")
            nc.sync.dma_start(msk_t[:], cmask)

            shuf_mask = list(range(16, 32)) + list(range(16))
            nc.gpsimd.memset(qz[0:DH, :, 1, :], 0.0)
            nc.gpsimd.memset(qz[DH:P, :, 0, :], 0.0)

            def proj(wt, dst, mb):
                for half in range(2):
                    ps = pr_ps.tile([P, S // 2], f32, tag="proj",
                                    name=f"pj{mb}_{half}")
                    for i in range(NEC):
                        for qt in range(2):
                            c0 = half * 1024 + qt * 512
                            nc.tensor.matmul(
                                ps[:, qt * 512:(qt + 1) * 512],
                                lhsT=wt[:, i, mb * P:(mb + 1) * P],
                                rhs=xT[:, i, c0:c0 + 512],
                                start=(i == 0),
                                stop=(i == NEC - 1),
                            )
                    if half == 0:
                        nc.vector.tensor_copy(
                            dst[:, mb, half * 1024:(half + 1) * 1024], ps[:]
                        )
                    else:
                        nc.scalar.copy(
                            dst[:, mb, half * 1024:(half + 1) * 1024], ps[:]
                        )

            def rope_k(mb, eng):
                # eng=gpsimd offloads the combine off DVE; all APs are
                # full-width base-0 (gpsimd mishandles partition offsets)
                sh = rtmp.tile([P, S], bf16, tag="shuf", name=f"shk{mb}")
                nc.vector.stream_shuffle(sh[:], kcT[:, mb, :], shuf_mask)
                eng.tensor_mul(sh[:], sh[:], sin_t[:])
                eng.tensor_mul(kT[:, mb, :], kcT[:, mb, :], cos_t[:])
                eng.tensor_add(kT[:, mb, :], kT[:, mb, :], sh[:])

            def rope_q(mb):
                sh = rtmp.tile([P, S], bf16, tag="shuf", name=f"shq{mb}")
                nc.vector.stream_shuffle(sh[:], qcT[:, mb, :], shuf_mask)
                nc.vector.tensor_mul(sh[:], sh[:], sin_t[:])
                for par in range(2):
                    o0 = par * DH
                    nc.vector.tensor_mul(
                        qz[o0:o0 + DH, mb, par, :],
                        qcT[o0:o0 + DH, mb, :],
                        cos_t[o0:o0 + DH, :],
                    )
                    nc.vector.tensor_add(
                        qz[o0:o0 + DH, mb, par, :],
                        qz[o0:o0 + DH, mb, par, :],
                        sh[o0:o0 + DH, :],
                    )

            proj(wk_t, kcT, 0)
            rope_k(0, nc.vector)
            proj(wq_t, qcT, 0)
            rope_q(0)
            proj(wk_t, kcT, 1)
            rope_k(1, nc.gpsimd)
            proj(wq_t, qcT, 1)
            rope_q(1)
            proj(wv_t, vT, 0)
            proj(wv_t, vT, 1)

            # V natural layout (t on partitions) + ones column per head
            nc.vector.memset(vn[:, :, :, 64:65], 1.0)
            for mb in range(MB):
                for sb_i in range(NSB):
                    ps = tp_ps.tile([P, P], bf16, tag="tp")
                    nc.tensor.transpose(
                        ps[:], vT[:, mb, sb_i * P:(sb_i + 1) * P], ident[:]
                    )
                    nc.vector.tensor_copy(
                        vn[:, sb_i, 2 * mb:2 * mb + 2, 0:64],
                        ps[:].rearrange("p (a b) -> p a b", a=2),
                    )

        # ---- Phase B: attention, two heads interleaved, q in halves ----
        # PSUM budget: 2 acc tiles (65, 1024) = 2 banks each + 2 sc bufs
        # (128, 1024) = 2 banks each -> 8 banks. Interleaving a head pair
        # keeps TensorE dense enough that HAM stays at full clock while
        # ACT runs the exps.
        with ExitStack() as bctx:
            sc_ps = bctx.enter_context(
                tc.tile_pool(name="sc_ps", bufs=2, space="PSUM")
            )
            ac_ps = bctx.enter_context(
                tc.tile_pool(name="ac_ps", bufs=1, space="PSUM")
            )
            ptp = bctx.enter_context(tc.tile_pool(name="ptp", bufs=4))
            dn = bctx.enter_context(tc.tile_pool(name="dn", bufs=2))

            deferred_norms = []
            for hp in range(2):
                heads = (2 * hp, 2 * hp + 1)
                for pss in range(2):
                    q0 = pss * 1024
                    accs = {
                        h: ac_ps.tile([65, 1024], f32, tag=f"acc{h % 2}",
                                      name=f"acc_{h}_{pss}")
                        for h in heads
                    }
                    def issue_pv(h, ti, pt, lo, hi):
                        # one PV piece per PSUM bank; bank bk (global)
                        # is complete at ti == 4*bk+3
                        p0 = lo
                        while p0 < hi:
                            bk = p0 // 512
                            p1 = min(hi, (bk + 1) * 512)
                            nc.tensor.matmul(
                                accs[h][:, p0 - q0:p1 - q0],
                                lhsT=vn[:, ti, h, :],
                                rhs=pt[:, p0 - q0:p1 - q0],
                                start=(ti == 0),
                                stop=(ti == 4 * bk + 3),
                            )
                            p0 = p1

                    # software pipeline: PV consumes the PREVIOUS
                    # iteration's exp output, so TensorE never waits on
                    # ScalarE inside an iteration (keeps the PE dense ->
                    # HAM stays at full clock; exp overlaps fully).
                    pending = []
                    for ti in range(8 if pss == 0 else NSB):
                        if ti == 2 and deferred_norms:
                            # previous pass's normalize chains, emitted
                            # here so the PSUM-release semaphores (which
                            # gate this pass's PV accumulators) are not
                            # queued behind ~8us of reciprocal work
                            for st in deferred_norms:
                                epilogue_norm(*st)
                            deferred_norms = []
                        if pss == 1 and ti == 13:
                            # PSUM bank 2 (cols q0..q0+512) got its last
                            # PV at ti==11: normalize it mid-loop while
                            # ti 13..15 still stream
                            for h2 in heads:
                                epilogue_norm(*epilogue_copies(h2, q0, q0 + 512))
                        t0 = ti * P
                        lo = max(t0, q0)
                        hi = q0 + 1024
                        new = []
                        scs = {}
                        for h in heads:
                            scs[h] = sc_ps.tile([P, 1024], f32, tag="sc",
                                                name=f"sc_{h}_{ti}")
                        p0 = lo
                        while p0 < hi:
                            p1 = min(hi, (p0 // 512 + 1) * 512)
                            for h in heads:
                                mb = h // 2
                                nc.tensor.matmul(
                                    scs[h][:, p0 - q0:p1 - q0],
                                    lhsT=kT[:, mb, t0:t0 + P],
                                    rhs=qz[:, mb, h % 2, p0:p1],
                                )
                            p0 = p1
                        for h in heads:
                            mb, off = h // 2, (h % 2) * DH
                            sc = scs[h]
                            pt = ptp.tile([P, 1024], bf16, tag="pt")
                            nc.scalar.activation(
                                pt[:, lo - q0:hi - q0],
                                sc[:, lo - q0:hi - q0],
                                AF.Exp,
                                scale=ATTN_SCALE,
                            )
                            if t0 >= q0:
                                nc.vector.tensor_mul(
                                    pt[:, t0 - q0:t0 - q0 + P],
                                    pt[:, t0 - q0:t0 - q0 + P],
                                    msk_t[:],
                                )
                            new.append((h, ti, pt, lo, hi))
                        for args in pending:
                            issue_pv(*args)
                        pending = new
                    for args in pending:
                        issue_pv(*args)
                    def epilogue_copies(h, c0, c1):
                        # stage out^T + the denominator row out of PSUM
                        # (plain DVE copies; custom-DVE ops mishandle
                        # PSUM/partition-offset inputs on HW). These two
                        # copies are all that holds the accumulator
                        # banks.
                        w = c1 - c0
                        l0 = c0 - q0
                        acb = dn.tile([DH, w], f32, tag=f"acb{h % 2}",
                                      name=f"acb{h}_{c0}")
                        nc.vector.tensor_copy(acb[:], accs[h][0:DH, l0:l0 + w])
                        den0 = dn.tile([1, w], f32, tag=f"den0{h % 2}",
                                       name=f"den0{h}_{c0}")
                        nc.vector.tensor_copy(
                            den0[:], accs[h][64:65, l0:l0 + w]
                        )
                        return h, c0, c1, acb, den0

                    def epilogue_norm(h, c0, c1, acb, den0):
                        # 2-pass approximate reciprocal (~22 bits), a
                        # partition broadcast on the (idle) GpSimd
                        # engine (NOTE: partition_broadcast on HW always
                        # reads the tile's partition 0, so rden must be
                        # a base-0 tile), then one multiply.
                        mb, off = h // 2, (h % 2) * DH
                        w = c1 - c0
                        rden = dn.tile([1, w], f32, tag=f"rden{h % 2}",
                                       name=f"rden{h}_{c0}")
                        rscr = dn.tile([1, w], f32, tag=f"rscr{h % 2}",
                                       name=f"rscr{h}_{c0}")
                        nc.vector.reciprocal_approx_accurate(
                            rden[:], den0[:], rscr[:]
                        )
                        rdb = dn.tile([DH, w], f32, tag="rdb",
                                      name=f"rdb{h}_{c0}")
                        nc.gpsimd.partition_broadcast(rdb[:], rden[:])
                        if debug:
                            nc.sync.dma_start(dbg["dacc"][:, h, c0:c1], acb[:])
                            nc.sync.dma_start(
                                dbg["dden"][:, h, c0:c1], rden[:]
                            )
                        nc.vector.tensor_mul(
                            onrm[off:off + DH, mb, c0:c1], acb[:], rdb[:]
                        )

                    if pss == 1:
                        chunks = [(h, q0 + 512, q0 + 1024) for h in heads]
                    else:
                        chunks = [(h, q0, q0 + 1024) for h in heads]
                    # stage 1 now (the PSUM-freeing copies); the
                    # reciprocal chains are deferred into the next
                    # pass's loop so the release semaphores on the DVE
                    # queue fire right after the copies.
                    staged = [epilogue_copies(*c) for c in chunks]
                    if hp == 1 and pss == 1:
                        for st in staged:
                            epilogue_norm(*st)
                    else:
                        deferred_norms = staged

        if debug:
            nc.sync.dma_start(dbg["dxT"], xT[:])
            nc.sync.dma_start(dbg["dqcT"], qcT[:])
            for _mb in range(MB):
                for _par in range(2):
                    _o0 = _par * DH
                    nc.sync.dma_start(
                        dbg["dqT"][_o0:_o0 + DH, _mb, :],
                        qz[_o0:_o0 + DH, _mb, _par, :],
                    )
            nc.sync.dma_start(dbg["dkT"], kT[:])
            nc.sync.dma_start(dbg["dvn"], vn[:])
            nc.sync.dma_start(dbg["donrm"], onrm[:])

        # ---- Phase C: output projection ----
        with ExitStack() as cctx:
            y_ps = cctx.enter_context(
                tc.tile_pool(name="y_ps", bufs=2, space="PSUM")
            )
            yo = cctx.enter_context(tc.tile_pool(name="yo", bufs=3))
            for sb_i in range(NSB):
                yp = y_ps.tile([P, E], f32, tag="yp")
                for mb in range(MB):
                    for half in range(2):
                        nc.tensor.matmul(
                            yp[:, half * 512:(half + 1) * 512],
                            lhsT=onrm[:, mb, sb_i * P:(sb_i + 1) * P],
                            rhs=wo_t[:, mb, half * 512:(half + 1) * 512],
                            start=(mb == 0),
                            stop=(mb == MB - 1),
                        )
                ys = yo.tile([P, E], f32, tag="ys")
                for half in range(2):
                    sl = slice(half * 512, (half + 1) * 512)
                    if (sb_i + half) % 2 == 0:
                        nc.vector.tensor_copy(ys[:, sl], yp[:, sl])
                    else:
                        nc.scalar.copy(ys[:, sl], yp[:, sl])
                    eng = nc.sync if half == 0 else nc.scalar
                    eng.dma_start(y[sb_i * P:(sb_i + 1) * P, sl], ys[:, sl])

    nc.compile()
    return nc


def get_program():
    global _PROG
    if _PROG is None:
        _PROG = _build_program()
    return _PROG


def make_in_maps(x, W_q, W_k, W_v, W_o):
    perm = _perm64()
    idx_local = (np.arange(DC) // 64) * 64 + perm[np.arange(DC) % 64]
    ang, sgn = _cos_sin_tiles()
    cos_np = np.cos(ang).astype(BF16)
    sin_np = (sgn * np.sin(ang)).astype(BF16)
    # scores tile is (t, q): keep t <= q -> upper triangular incl. diagonal
    cmask_np = np.triu(np.ones((P, P))).astype(BF16)
    in_maps = []
    for c in range(NCORES):
        b, hg = c // 4, c % 4
        base = hg * DC
        in_maps.append(
            dict(
                xbt=np.ascontiguousarray(x[b].T.astype(BF16)),
                wq=np.ascontiguousarray(W_q[:, base + idx_local].astype(BF16)),
                wk=np.ascontiguousarray(W_k[:, base + idx_local].astype(BF16)),
                wv=np.ascontiguousarray(W_v[:, base:base + DC].astype(BF16)),
                wo=np.ascontiguousarray(W_o[base:base + DC, :].astype(BF16)),
                cosr=cos_np,
                sinr=sin_np,
                cmask=cmask_np,
            )
        )
    return in_maps


def kernel(x, W_q, W_k, W_v, W_o, _trace=False, _trace_cores=None):
    from concourse.bass_utils import run_bass_kernel_spmd

    x = np.asarray(x, dtype=np.float32)
    W_q = np.asarray(W_q, dtype=np.float32)
    W_k = np.asarray(W_k, dtype=np.float32)
    W_v = np.asarray(W_v, dtype=np.float32)
    W_o = np.asarray(W_o, dtype=np.float32)

    nc = get_program()
    in_maps = make_in_maps(x, W_q, W_k, W_v, W_o)
    res = run_bass_kernel_spmd(
        nc,
        in_maps,
        list(range(NCORES)),
        trace=_trace,
        trace_cores=_trace_cores,
    )
    y = np.zeros((B, S, E), np.float32)
    for c in range(NCORES):
        y[c // 4] += res.results[c]["y"]
    if _trace:
        return y, res
    return y


# revision 34
# speedup vs baseline: 1.0423x; 1.0423x over previous
"""Multi-head causal attention with RoPE on 8 Trainium2 NeuronCores.

Sharding: data-parallel over batch (B=2) x tensor-parallel over heads
(16 heads -> 4 groups of 4). Core c handles batch c//4, heads
[(c%4)*4, (c%4)*4+4). Each core computes a partial y = attn_out @ W_o
for its head group; the host sums the 4 partials per batch (the "W_o
all-reduce").

Device kernel (per core, all matmuls bf16, fp32 PSUM accumulation):
  - x^T built on-chip via PE transposes (contraction over E needs E on
    partitions).
  - Q^T/K^T/V^T projections in "T layout" (dims on partitions, seq on
    free): out = W_chunk.T @ x^T_chunk accumulated over 8 E-chunks.
  - RoPE: the within-head pair shuffle is folded into a host-side
    permutation of W_q/W_k columns so the rotation partner sits 16
    partitions away inside the same 32-partition quadrant; on device a
    single DVE stream_shuffle + cos/sin multiply-adds apply the
    rotation. Scores are permutation-invariant since Q and K use the
    same permutation.
  - scores^T[t, q] = K^T_tile.T @ Q^T (only t-blocks <= q-block:
    causal skip), exp on ACT (scale=1/32 folded in), causal mask on
    diagonal blocks, P^T @ [V | 1] accumulated in PSUM -> out^T plus
    softmax denominators in one matmul (ones column appended to V).
  - normalize with reciprocal + gpsimd partition_broadcast, then
    y = out_norm^T.T @ W_o chunks.
"""

import os
import sys
from contextlib import ExitStack

import numpy as np

for _p in ("/opt/trn_rl_repo",):
    if os.path.isdir(_p) and _p not in sys.path:
        sys.path.insert(0, _p)

import ml_dtypes  # noqa: E402

BF16 = ml_dtypes.bfloat16

B, S, E = 2, 2048, 1024
H, DH = 16, 64
NCORES = 8
HPC = H // 4          # 4 heads per core
DC = HPC * DH         # 256 head dims per core
ATTN_SCALE = 1.0 / 32.0  # 1/sqrt(E)
ROPE_BASE = 10000.0
P = 128
NSB = S // P          # 16 sequence blocks
NEC = E // P          # 8 E chunks
MB = DC // P          # 2 partition blocks of head dims

_PROG = None


def _perm64():
    """perm[j] = original head-dim index stored at permuted position j.

    Quadrant q of the permuted layout holds RoPE pairs i in
    [16q, 16q+16): even elements (2i) at slots 0-15, odd (2i+1) at
    slots 16-31. The rotation partner is then always +-16 partitions
    away within one 32-partition quadrant (stream_shuffle range).
    """
    j = np.arange(64)
    qd, r = j // 32, j % 32
    i = 16 * qd + (r % 16)
    return 2 * i + (r >= 16)


def _cos_sin_tiles():
    pl = np.arange(P) % 64
    qd, r = pl // 32, pl % 32
    i = 16 * qd + (r % 16)
    inv = ROPE_BASE ** (-(2.0 * i) / DH)
    ang = np.arange(S)[None, :] * inv[:, None]          # (128, S)
    sgn = np.where(r < 16, -1.0, 1.0)[:, None]
    return ang, sgn


def _build_program(debug=False):
    import concourse.bacc as bacc
    import concourse.tile as tile
    from concourse import masks, mybir

    f32 = mybir.dt.float32
    bf16 = mybir.dt.bfloat16
    AF = mybir.ActivationFunctionType

    nc = bacc.Bacc("TRN2", target_bir_lowering=False, debug=False)
    xbt = nc.dram_tensor("xbt", [E, S], bf16, kind="ExternalInput").ap()
    wq = nc.dram_tensor("wq", [E, DC], bf16, kind="ExternalInput").ap()
    wk = nc.dram_tensor("wk", [E, DC], bf16, kind="ExternalInput").ap()
    wv = nc.dram_tensor("wv", [E, DC], bf16, kind="ExternalInput").ap()
    wo = nc.dram_tensor("wo", [DC, E], bf16, kind="ExternalInput").ap()
    cosr = nc.dram_tensor("cosr", [P, S], bf16, kind="ExternalInput").ap()
    sinr = nc.dram_tensor("sinr", [P, S], bf16, kind="ExternalInput").ap()
    cmask = nc.dram_tensor("cmask", [P, P], bf16, kind="ExternalInput").ap()
    y = nc.dram_tensor("y", [S, E], f32, kind="ExternalOutput").ap()
    if debug:
        dbg = {
            "dxT": nc.dram_tensor("dxT", [P, NEC, S], mybir.dt.bfloat16,
                                  kind="ExternalOutput").ap(),
            "dqcT": nc.dram_tensor("dqcT", [P, MB, S], mybir.dt.bfloat16,
                                   kind="ExternalOutput").ap(),
            "dqT": nc.dram_tensor("dqT", [P, MB, S], mybir.dt.bfloat16,
                                  kind="ExternalOutput").ap(),
            "dkT": nc.dram_tensor("dkT", [P, MB, S], mybir.dt.bfloat16,
                                  kind="ExternalOutput").ap(),
            "dvn": nc.dram_tensor("dvn", [P, NSB, HPC, 65], mybir.dt.bfloat16,
                                  kind="ExternalOutput").ap(),
            "donrm": nc.dram_tensor("donrm", [P, MB, S], mybir.dt.bfloat16,
                                    kind="ExternalOutput").ap(),
            "dacc": nc.dram_tensor("dacc", [DH, HPC, S], f32,
                                   kind="ExternalOutput").ap(),
            "dden": nc.dram_tensor("dden", [1, HPC, S], f32,
                                   kind="ExternalOutput").ap(),
        }

    with ExitStack() as ctx:
        tc = ctx.enter_context(tile.TileContext(nc))
        consts = ctx.enter_context(tc.tile_pool(name="consts", bufs=1))
        persist = ctx.enter_context(tc.tile_pool(name="persist", bufs=1))

        ident = consts.tile([P, P], bf16, tag="ident")
        masks.make_identity(nc, ident[:])

        qcT = persist.tile([P, MB, S], bf16, tag="qcT")
        kcT = persist.tile([P, MB, S], bf16, tag="kcT")
        vT = persist.tile([P, MB, S], bf16, tag="vT")
        # qz holds RoPE'd Q^T zero-padded per head parity: slice
        # [:, mb, par, :] has head (2*mb+par)'s 64 rows live and the
        # other 64 rows zero. Scores then use the FULL 128-row K^T
        # block as lhsT (K=128): the HAM clock gate never grants full
        # clock to half-height (K=64) matmul streams, and the zero
        # rows contribute exactly 0.
        qz = persist.tile([P, MB, 2, S], bf16, tag="qz")
        kT = persist.tile([P, MB, S], bf16, tag="kT")
        vn = persist.tile([P, NSB, HPC, 65], bf16, tag="vn")
        onrm = persist.tile([P, MB, S], bf16, tag="onrm")

        # ---- Phase A: x^T, projections, RoPE, V natural ----
        with ExitStack() as actx:
            xnat = actx.enter_context(tc.tile_pool(name="xnat", bufs=3))
            tp_ps = actx.enter_context(
                tc.tile_pool(name="tp_ps", bufs=3, space="PSUM")
            )
            pr_ps = actx.enter_context(
                tc.tile_pool(name="pr_ps", bufs=2, space="PSUM")
            )
            rtmp = actx.enter_context(tc.tile_pool(name="rtmp", bufs=2))
            xp = actx.enter_context(tc.tile_pool(name="xp", bufs=1))
            xT = xp.tile([P, NEC, S], bf16, tag="xT")

            # K-projection weights first (the first matmul group), then
            # x^T chunks interleaved across both HWDGE rings.
            wk_t = consts.tile([P, NEC, DC], bf16, tag="wk")
            nc.scalar.dma_start(wk_t[:], wk.rearrange("(c p) m -> p c m", p=P))
            for ec in range(NEC):
                eng = nc.sync if ec % 2 == 0 else nc.scalar
                eng.dma_start(xT[:, ec, :], xbt[ec * P:(ec + 1) * P, :])
            wq_t = consts.tile([P, NEC, DC], bf16, tag="wq")
            nc.sync.dma_start(wq_t[:], wq.rearrange("(c p) m -> p c m", p=P))
            wv_t = consts.tile([P, NEC, DC], bf16, tag="wv")
            nc.scalar.dma_start(wv_t[:], wv.rearrange("(c p) m -> p c m", p=P))
            cos_t = consts.tile([P, S], bf16, tag="cos")
            nc.sync.dma_start(cos_t[:], cosr)
            sin_t = consts.tile([P, S], bf16, tag="sin")
            nc.scalar.dma_start(sin_t[:], sinr)
            wo_t = consts.tile([P, MB, E], bf16, tag="wo")
            nc.sync.dma_start(wo_t[:], wo.rearrange("(c p) n -> p c n", p=P))
            msk_t = consts.tile([P, P], bf16, tag="msk")
            nc.scalar.dma_start(msk_t[:], cmask)

            shuf_mask = list(range(16, 32)) + list(range(16))
            nc.gpsimd.memset(qz[0:DH, :, 1, :], 0.0)
            nc.gpsimd.memset(qz[DH:P, :, 0, :], 0.0)

            def proj(wt, dst, mb):
                for half in range(2):
                    ps = pr_ps.tile([P, S // 2], f32, tag="proj",
                                    name=f"pj{mb}_{half}")
                    for i in range(NEC):
                        for qt in range(2):
                            c0 = half * 1024 + qt * 512
                            nc.tensor.matmul(
                                ps[:, qt * 512:(qt + 1) * 512],
                                lhsT=wt[:, i, mb * P:(mb + 1) * P],
                                rhs=xT[:, i, c0:c0 + 512],
                                start=(i == 0),
                                stop=(i == NEC - 1),
                            )
                    if half == 0:
                        nc.vector.tensor_copy(
                            dst[:, mb, half * 1024:(half + 1) * 1024], ps[:]
                        )
                    else:
                        nc.scalar.copy(
                            dst[:, mb, half * 1024:(half + 1) * 1024], ps[:]
                        )

            def rope_k(mb, eng, pool):
                # eng=gpsimd offloads the combine off DVE; all APs are
                # full-width base-0 (gpsimd mishandles partition offsets)
                sh = pool.tile([P, S], bf16, tag="shuf", name=f"shk{mb}")
                nc.vector.stream_shuffle(sh[:], kcT[:, mb, :], shuf_mask)
                eng.tensor_mul(sh[:], sh[:], sin_t[:])
                eng.tensor_mul(kT[:, mb, :], kcT[:, mb, :], cos_t[:])
                eng.tensor_add(kT[:, mb, :], kT[:, mb, :], sh[:])

            def rope_q(mb, pool):
                sh = pool.tile([P, S], bf16, tag="shuf", name=f"shq{mb}")
                nc.vector.stream_shuffle(sh[:], qcT[:, mb, :], shuf_mask)
                nc.vector.tensor_mul(sh[:], sh[:], sin_t[:])
                for par in range(2):
                    o0 = par * DH
                    nc.vector.tensor_mul(
                        qz[o0:o0 + DH, mb, par, :],
                        qcT[o0:o0 + DH, mb, :],
                        cos_t[o0:o0 + DH, :],
                    )
                    nc.vector.tensor_add(
                        qz[o0:o0 + DH, mb, par, :],
                        qz[o0:o0 + DH, mb, par, :],
                        sh[o0:o0 + DH, :],
                    )

            proj(wk_t, kcT, 0)
            rope_k(0, nc.vector, rtmp)
            proj(wq_t, qcT, 0)
            rope_q(0, rtmp)
            proj(wk_t, kcT, 1)
            proj(wq_t, qcT, 1)
            proj(wv_t, vT, 0)
            proj(wv_t, vT, 1)
            rope_fns = (rope_k, rope_q)

            # V natural layout (t on partitions) + ones column per head
            nc.vector.memset(vn[:, :, :, 64:65], 1.0)
            for mb in range(MB):
                for sb_i in range(NSB):
                    ps = tp_ps.tile([P, P], bf16, tag="tp")
                    nc.tensor.transpose(
                        ps[:], vT[:, mb, sb_i * P:(sb_i + 1) * P], ident[:]
                    )
                    nc.vector.tensor_copy(
                        vn[:, sb_i, 2 * mb:2 * mb + 2, 0:64],
                        ps[:].rearrange("p (a b) -> p a b", a=2),
                    )

        # ---- Phase B: attention, two heads interleaved, q in halves ----
        # PSUM budget: 2 acc tiles (65, 1024) = 2 banks each + 2 sc bufs
        # (128, 1024) = 2 banks each -> 8 banks. Interleaving a head pair
        # keeps TensorE dense enough that HAM stays at full clock while
        # ACT runs the exps.
        with ExitStack() as bctx:
            sc_ps = bctx.enter_context(
                tc.tile_pool(name="sc_ps", bufs=2, space="PSUM")
            )
            ac_ps = bctx.enter_context(
                tc.tile_pool(name="ac_ps", bufs=1, space="PSUM")
            )
            ptp = bctx.enter_context(tc.tile_pool(name="ptp", bufs=4))
            dn = bctx.enter_context(tc.tile_pool(name="dn", bufs=2))

            # mb=1 RoPE runs here so it doesn't gate attention start
            # (the phase-B pools only open once ALL phase-A work is
            # done); its outputs are first needed mid-attention by the
            # second head pair.
            rope_fns[0](1, nc.gpsimd, dn)
            rope_fns[1](1, dn)

            deferred_norms = []
            for hp in range(2):
                heads = (2 * hp, 2 * hp + 1)
                for pss in range(2):
                    q0 = pss * 1024
                    accs = {
                        h: ac_ps.tile([65, 1024], f32, tag=f"acc{h % 2}",
                                      name=f"acc_{h}_{pss}")
                        for h in heads
                    }
                    def issue_pv(h, ti, pt, lo, hi):
                        # one PV piece per PSUM bank; bank bk (global)
                        # is complete at ti == 4*bk+3
                        p0 = lo
                        while p0 < hi:
                            bk = p0 // 512
                            p1 = min(hi, (bk + 1) * 512)
                            nc.tensor.matmul(
                                accs[h][:, p0 - q0:p1 - q0],
                                lhsT=vn[:, ti, h, :],
                                rhs=pt[:, p0 - q0:p1 - q0],
                                start=(ti == 0),
                                stop=(ti == 4 * bk + 3),
                            )
                            p0 = p1

                    # software pipeline: PV consumes the PREVIOUS
                    # iteration's exp output, so TensorE never waits on
                    # ScalarE inside an iteration (keeps the PE dense ->
                    # HAM stays at full clock; exp overlaps fully).
                    pending = []
                    for ti in range(8 if pss == 0 else NSB):
                        if ti == 2 and deferred_norms:
                            # previous pass's normalize chains, emitted
                            # here so the PSUM-release semaphores (which
                            # gate this pass's PV accumulators) are not
                            # queued behind ~8us of reciprocal work
                            for st in deferred_norms:
                                epilogue_norm(*st)
                            deferred_norms = []
                        if pss == 1 and ti == 13:
                            # PSUM bank 2 (cols q0..q0+512) got its last
                            # PV at ti==11: normalize it mid-loop while
                            # ti 13..15 still stream
                            for h2 in heads:
                                epilogue_norm(*epilogue_copies(h2, q0, q0 + 512))
                        t0 = ti * P
                        lo = max(t0, q0)
                        hi = q0 + 1024
                        new = []
                        scs = {}
                        for h in heads:
                            scs[h] = sc_ps.tile([P, 1024], f32, tag="sc",
                                                name=f"sc_{h}_{ti}")
                        p0 = lo
                        while p0 < hi:
                            p1 = min(hi, (p0 // 512 + 1) * 512)
                            for h in heads:
                                mb = h // 2
                                nc.tensor.matmul(
                                    scs[h][:, p0 - q0:p1 - q0],
                                    lhsT=kT[:, mb, t0:t0 + P],
                                    rhs=qz[:, mb, h % 2, p0:p1],
                                )
                            p0 = p1
                        for h in heads:
                            mb, off = h // 2, (h % 2) * DH
                            sc = scs[h]
                            pt = ptp.tile([P, 1024], bf16, tag="pt")
                            nc.scalar.activation(
                                pt[:, lo - q0:hi - q0],
                                sc[:, lo - q0:hi - q0],
                                AF.Exp,
                                scale=ATTN_SCALE,
                            )
                            if t0 >= q0:
                                nc.vector.tensor_mul(
                                    pt[:, t0 - q0:t0 - q0 + P],
                                    pt[:, t0 - q0:t0 - q0 + P],
                                    msk_t[:],
                                )
                            new.append((h, ti, pt, lo, hi))
                        for args in pending:
                            issue_pv(*args)
                        pending = new
                    for args in pending:
                        issue_pv(*args)
                    def epilogue_copies(h, c0, c1):
                        # stage out^T + the denominator row out of PSUM
                        # (plain DVE copies; custom-DVE ops mishandle
                        # PSUM/partition-offset inputs on HW). These two
                        # copies are all that holds the accumulator
                        # banks.
                        w = c1 - c0
                        l0 = c0 - q0
                        acb = dn.tile([DH, w], f32, tag=f"acb{h % 2}",
                                      name=f"acb{h}_{c0}")
                        nc.vector.tensor_copy(acb[:], accs[h][0:DH, l0:l0 + w])
                        den0 = dn.tile([1, w], f32, tag=f"den0{h % 2}",
                                       name=f"den0{h}_{c0}")
                        nc.vector.tensor_copy(
                            den0[:], accs[h][64:65, l0:l0 + w]
                        )
                        return h, c0, c1, acb, den0

                    def epilogue_norm(h, c0, c1, acb, den0):
                        # 2-pass approximate reciprocal (~22 bits), a
                        # partition broadcast on the (idle) GpSimd
                        # engine (NOTE: partition_broadcast on HW always
                        # reads the tile's partition 0, so rden must be
                        # a base-0 tile), then one multiply.
                        mb, off = h // 2, (h % 2) * DH
                        w = c1 - c0
                        rden = dn.tile([1, w], f32, tag=f"rden{h % 2}",
                                       name=f"rden{h}_{c0}")
                        rscr = dn.tile([1, w], f32, tag=f"rscr{h % 2}",
                                       name=f"rscr{h}_{c0}")
                        nc.vector.reciprocal_approx_accurate(
                            rden[:], den0[:], rscr[:]
                        )
                        rdb = dn.tile([DH, w], f32, tag="rdb",
                                      name=f"rdb{h}_{c0}")
                        nc.gpsimd.partition_broadcast(rdb[:], rden[:])
                        if debug:
                            nc.sync.dma_start(dbg["dacc"][:, h, c0:c1], acb[:])
                            nc.sync.dma_start(
                                dbg["dden"][:, h, c0:c1], rden[:]
                            )
                        nc.vector.tensor_mul(
                            onrm[off:off + DH, mb, c0:c1], acb[:], rdb[:]
                        )

                    if pss == 1:
                        chunks = [(h, q0 + 512, q0 + 1024) for h in heads]
                    else:
                        chunks = [(h, q0, q0 + 1024) for h in heads]
                    # stage 1 now (the PSUM-freeing copies); the
                    # reciprocal chains are deferred into the next
                    # pass's loop so the release semaphores on the DVE
                    # queue fire right after the copies.
                    staged = [epilogue_copies(*c) for c in chunks]
                    if hp == 1 and pss == 1:
                        for st in staged:
                            epilogue_norm(*st)
                    else:
                        deferred_norms = staged

        if debug:
            nc.sync.dma_start(dbg["dxT"], xT[:])
            nc.sync.dma_start(dbg["dqcT"], qcT[:])
            for _mb in range(MB):
                for _par in range(2):
                    _o0 = _par * DH
                    nc.sync.dma_start(
                        dbg["dqT"][_o0:_o0 + DH, _mb, :],
                        qz[_o0:_o0 + DH, _mb, _par, :],
                    )
            nc.sync.dma_start(dbg["dkT"], kT[:])
            nc.sync.dma_start(dbg["dvn"], vn[:])
            nc.sync.dma_start(dbg["donrm"], onrm[:])

        # ---- Phase C: output projection ----
        with ExitStack() as cctx:
            y_ps = cctx.enter_context(
                tc.tile_pool(name="y_ps", bufs=2, space="PSUM")
            )
            yo = cctx.enter_context(tc.tile_pool(name="yo", bufs=3))
            for sb_i in range(NSB):
                yp = y_ps.tile([P, E], f32, tag="yp")
                for mb in range(MB):
                    for half in range(2):
                        nc.tensor.matmul(
                            yp[:, half * 512:(half + 1) * 512],
                            lhsT=onrm[:, mb, sb_i * P:(sb_i + 1) * P],
                            rhs=wo_t[:, mb, half * 512:(half + 1) * 512],
                            start=(mb == 0),
                            stop=(mb == MB - 1),
                        )
                ys = yo.tile([P, E], f32, tag="ys")
                for half in range(2):
                    sl = slice(half * 512, (half + 1) * 512)
                    if (sb_i + half) % 2 == 0:
                        nc.vector.tensor_copy(ys[:, sl], yp[:, sl])
                    else:
                        nc.scalar.copy(ys[:, sl], yp[:, sl])
                    eng = nc.sync if half == 0 else nc.scalar
                    eng.dma_start(y[sb_i * P:(sb_i + 1) * P, sl], ys[:, sl])

    nc.compile()
    return nc


def get_program():
    global _PROG
    if _PROG is None:
        _PROG = _build_program()
    return _PROG


def make_in_maps(x, W_q, W_k, W_v, W_o):
    perm = _perm64()
    idx_local = (np.arange(DC) // 64) * 64 + perm[np.arange(DC) % 64]
    ang, sgn = _cos_sin_tiles()
    cos_np = np.cos(ang).astype(BF16)
    sin_np = (sgn * np.sin(ang)).astype(BF16)
    # scores tile is (t, q): keep t <= q -> upper triangular incl. diagonal
    cmask_np = np.triu(np.ones((P, P))).astype(BF16)
    in_maps = []
    for c in range(NCORES):
        b, hg = c // 4, c % 4
        base = hg * DC
        in_maps.append(
            dict(
                xbt=np.ascontiguousarray(x[b].T.astype(BF16)),
                wq=np.ascontiguousarray(W_q[:, base + idx_local].astype(BF16)),
                wk=np.ascontiguousarray(W_k[:, base + idx_local].astype(BF16)),
                wv=np.ascontiguousarray(W_v[:, base:base + DC].astype(BF16)),
                wo=np.ascontiguousarray(W_o[base:base + DC, :].astype(BF16)),
                cosr=cos_np,
                sinr=sin_np,
                cmask=cmask_np,
            )
        )
    return in_maps


def kernel(x, W_q, W_k, W_v, W_o, _trace=False, _trace_cores=None):
    from concourse.bass_utils import run_bass_kernel_spmd

    x = np.asarray(x, dtype=np.float32)
    W_q = np.asarray(W_q, dtype=np.float32)
    W_k = np.asarray(W_k, dtype=np.float32)
    W_v = np.asarray(W_v, dtype=np.float32)
    W_o = np.asarray(W_o, dtype=np.float32)

    nc = get_program()
    in_maps = make_in_maps(x, W_q, W_k, W_v, W_o)
    res = run_bass_kernel_spmd(
        nc,
        in_maps,
        list(range(NCORES)),
        trace=_trace,
        trace_cores=_trace_cores,
    )
    y = np.zeros((B, S, E), np.float32)
    for c in range(NCORES):
        y[c // 4] += res.results[c]["y"]
    if _trace:
        return y, res
    return y


# revision 35
# speedup vs baseline: 1.0475x; 1.0050x over previous
"""Multi-head causal attention with RoPE on 8 Trainium2 NeuronCores.

Sharding: data-parallel over batch (B=2) x tensor-parallel over heads
(16 heads -> 4 groups of 4). Core c handles batch c//4, heads
[(c%4)*4, (c%4)*4+4). Each core computes a partial y = attn_out @ W_o
for its head group; the host sums the 4 partials per batch (the "W_o
all-reduce").

Device kernel (per core, all matmuls bf16, fp32 PSUM accumulation):
  - x^T comes pre-transposed from the host (contraction over E needs E
    on SBUF partitions; host-side transpose avoids slow strided xbar
    DMAs).
  - Q^T/K^T/V^T projections in "T layout" (dims on partitions, seq on
    free): out = W_chunk.T @ x^T_chunk accumulated over 8 E-chunks.
  - RoPE: the within-head pair shuffle is folded into a host-side
    permutation of W_q/W_k columns so the rotation partner sits 16
    partitions away inside the same 32-partition quadrant; on device a
    single DVE stream_shuffle + cos/sin multiply-adds apply the
    rotation. Scores are permutation-invariant since Q and K use the
    same permutation.
  - attention per head pair, q-range in two 1024-col passes (PSUM
    budget), PVs software-pipelined one iteration behind the scores:
    scores^T[t, q] = K^T_block.T @ Qz (full K=128 via zero-padded Q --
    the HAM clock gate never grants full clock to K=64 streams; only
    t-blocks <= q-block: causal skip), exp on ACT (scale=1/32 folded
    in), causal mask on diagonal blocks, P^T @ [V | 1] accumulated in
    PSUM -> out^T plus softmax denominators (ones column in V).
  - normalize: 2-pass approx reciprocal + gpsimd partition_broadcast
    (norm chains deferred a pass so PSUM-release sems fire early),
    then y = out_norm^T.T @ W_o chunks.
"""

import os
import sys
from contextlib import ExitStack

import numpy as np

for _p in ("/opt/trn_rl_repo",):
    if os.path.isdir(_p) and _p not in sys.path:
        sys.path.insert(0, _p)

import ml_dtypes  # noqa: E402

BF16 = ml_dtypes.bfloat16

B, S, E = 2, 2048, 1024
H, DH = 16, 64
NCORES = 8
HPC = H // 4          # 4 heads per core
DC = HPC * DH         # 256 head dims per core
ATTN_SCALE = 1.0 / 32.0  # 1/sqrt(E)
ROPE_BASE = 10000.0
P = 128
NSB = S // P          # 16 sequence blocks
NEC = E // P          # 8 E chunks
MB = DC // P          # 2 partition blocks of head dims

_PROG = None


def _perm64():
    """perm[j] = original head-dim index stored at permuted position j.

    Quadrant q of the permuted layout holds RoPE pairs i in
    [16q, 16q+16): even elements (2i) at slots 0-15, odd (2i+1) at
    slots 16-31. The rotation partner is then always +-16 partitions
    away within one 32-partition quadrant (stream_shuffle range).
    """
    j = np.arange(64)
    qd, r = j // 32, j % 32
    i = 16 * qd + (r % 16)
    return 2 * i + (r >= 16)


def _cos_sin_tiles():
    pl = np.arange(P) % 64
    qd, r = pl // 32, pl % 32
    i = 16 * qd + (r % 16)
    inv = ROPE_BASE ** (-(2.0 * i) / DH)
    ang = np.arange(S)[None, :] * inv[:, None]          # (128, S)
    sgn = np.where(r < 16, -1.0, 1.0)[:, None]
    return ang, sgn


def _build_program(debug=False):
    import concourse.bacc as bacc
    import concourse.tile as tile
    from concourse import masks, mybir

    f32 = mybir.dt.float32
    bf16 = mybir.dt.bfloat16
    AF = mybir.ActivationFunctionType

    nc = bacc.Bacc("TRN2", target_bir_lowering=False, debug=False)
    xbt = nc.dram_tensor("xbt", [E, S], bf16, kind="ExternalInput").ap()
    wq = nc.dram_tensor("wq", [E, DC], bf16, kind="ExternalInput").ap()
    wk = nc.dram_tensor("wk", [E, DC], bf16, kind="ExternalInput").ap()
    wv = nc.dram_tensor("wv", [E, DC], bf16, kind="ExternalInput").ap()
    wo = nc.dram_tensor("wo", [DC, E], bf16, kind="ExternalInput").ap()
    cosr = nc.dram_tensor("cosr", [P, S], bf16, kind="ExternalInput").ap()
    sinr = nc.dram_tensor("sinr", [P, S], bf16, kind="ExternalInput").ap()
    cmask = nc.dram_tensor("cmask", [P, P], bf16, kind="ExternalInput").ap()
    y = nc.dram_tensor("y", [S, E], f32, kind="ExternalOutput").ap()
    if debug:
        dbg = {
            "dxT": nc.dram_tensor("dxT", [P, NEC, S], mybir.dt.bfloat16,
                                  kind="ExternalOutput").ap(),
            "dqcT": nc.dram_tensor("dqcT", [P, MB, S], mybir.dt.bfloat16,
                                   kind="ExternalOutput").ap(),
            "dqT": nc.dram_tensor("dqT", [P, MB, S], mybir.dt.bfloat16,
                                  kind="ExternalOutput").ap(),
            "dkT": nc.dram_tensor("dkT", [P, MB, S], mybir.dt.bfloat16,
                                  kind="ExternalOutput").ap(),
            "dvn": nc.dram_tensor("dvn", [P, NSB, HPC, 65], mybir.dt.bfloat16,
                                  kind="ExternalOutput").ap(),
            "donrm": nc.dram_tensor("donrm", [P, MB, S], mybir.dt.bfloat16,
                                    kind="ExternalOutput").ap(),
            "dacc": nc.dram_tensor("dacc", [DH, HPC, S], f32,
                                   kind="ExternalOutput").ap(),
            "dden": nc.dram_tensor("dden", [1, HPC, S], f32,
                                   kind="ExternalOutput").ap(),
        }

    with ExitStack() as ctx:
        tc = ctx.enter_context(tile.TileContext(nc))
        consts = ctx.enter_context(tc.tile_pool(name="consts", bufs=1))
        persist = ctx.enter_context(tc.tile_pool(name="persist", bufs=1))

        ident = consts.tile([P, P], bf16, tag="ident")
        masks.make_identity(nc, ident[:])

        qcT = persist.tile([P, MB, S], bf16, tag="qcT")
        kcT = persist.tile([P, MB, S], bf16, tag="kcT")
        vT = persist.tile([P, MB, S], bf16, tag="vT")
        # qz holds RoPE'd Q^T zero-padded per head parity: slice
        # [:, mb, par, :] has head (2*mb+par)'s 64 rows live and the
        # other 64 rows zero. Scores then use the FULL 128-row K^T
        # block as lhsT (K=128): the HAM clock gate never grants full
        # clock to half-height (K=64) matmul streams, and the zero
        # rows contribute exactly 0.
        qz = persist.tile([P, MB, 2, S], bf16, tag="qz")
        kT = persist.tile([P, MB, S], bf16, tag="kT")
        vn = persist.tile([P, NSB, HPC, 65], bf16, tag="vn")
        onrm = persist.tile([P, MB, S], bf16, tag="onrm")

        # ---- Phase A: x^T, projections, RoPE, V natural ----
        with ExitStack() as actx:
            xnat = actx.enter_context(tc.tile_pool(name="xnat", bufs=3))
            tp_ps = actx.enter_context(
                tc.tile_pool(name="tp_ps", bufs=3, space="PSUM")
            )
            pr_ps = actx.enter_context(
                tc.tile_pool(name="pr_ps", bufs=2, space="PSUM")
            )
            rtmp = actx.enter_context(tc.tile_pool(name="rtmp", bufs=2))
            xp = actx.enter_context(tc.tile_pool(name="xp", bufs=1))
            xT = xp.tile([P, NEC, S], bf16, tag="xT")

            # K-projection weights first (the first matmul group), then
            # x^T chunks interleaved across both HWDGE rings.
            wk_t = consts.tile([P, NEC, DC], bf16, tag="wk")
            nc.scalar.dma_start(wk_t[:], wk.rearrange("(c p) m -> p c m", p=P))
            for ec in range(NEC):
                eng = nc.sync if ec % 2 == 0 else nc.scalar
                eng.dma_start(xT[:, ec, :], xbt[ec * P:(ec + 1) * P, :])
            wq_t = consts.tile([P, NEC, DC], bf16, tag="wq")
            nc.sync.dma_start(wq_t[:], wq.rearrange("(c p) m -> p c m", p=P))
            wv_t = consts.tile([P, NEC, DC], bf16, tag="wv")
            nc.scalar.dma_start(wv_t[:], wv.rearrange("(c p) m -> p c m", p=P))
            cos_t = consts.tile([P, S], bf16, tag="cos")
            nc.sync.dma_start(cos_t[:], cosr)
            sin_t = consts.tile([P, S], bf16, tag="sin")
            nc.scalar.dma_start(sin_t[:], sinr)
            wo_t = consts.tile([P, MB, E], bf16, tag="wo")
            nc.sync.dma_start(wo_t[:], wo.rearrange("(c p) n -> p c n", p=P))
            msk_t = consts.tile([P, P], bf16, tag="msk")
            nc.scalar.dma_start(msk_t[:], cmask)

            shuf_mask = list(range(16, 32)) + list(range(16))
            nc.gpsimd.memset(qz[0:DH, :, 1, :], 0.0)
            nc.gpsimd.memset(qz[DH:P, :, 0, :], 0.0)

            def proj(wt, dst, mb):
                for half in range(2):
                    ps = pr_ps.tile([P, S // 2], f32, tag="proj",
                                    name=f"pj{mb}_{half}")
                    for i in range(NEC):
                        for qt in range(2):
                            c0 = half * 1024 + qt * 512
                            nc.tensor.matmul(
                                ps[:, qt * 512:(qt + 1) * 512],
                                lhsT=wt[:, i, mb * P:(mb + 1) * P],
                                rhs=xT[:, i, c0:c0 + 512],
                                start=(i == 0),
                                stop=(i == NEC - 1),
                            )
                    if half == 0:
                        nc.vector.tensor_copy(
                            dst[:, mb, half * 1024:(half + 1) * 1024], ps[:]
                        )
                    else:
                        nc.scalar.copy(
                            dst[:, mb, half * 1024:(half + 1) * 1024], ps[:]
                        )

            def rope_k(mb, eng, pool):
                # eng=gpsimd offloads the combine off DVE; all APs are
                # full-width base-0 (gpsimd mishandles partition offsets)
                sh = pool.tile([P, S], bf16, tag="shuf", name=f"shk{mb}")
                nc.vector.stream_shuffle(sh[:], kcT[:, mb, :], shuf_mask)
                eng.tensor_mul(sh[:], sh[:], sin_t[:])
                eng.tensor_mul(kT[:, mb, :], kcT[:, mb, :], cos_t[:])
                eng.tensor_add(kT[:, mb, :], kT[:, mb, :], sh[:])

            def rope_q(mb, pool):
                sh = pool.tile([P, S], bf16, tag="shuf", name=f"shq{mb}")
                nc.vector.stream_shuffle(sh[:], qcT[:, mb, :], shuf_mask)
                nc.vector.tensor_mul(sh[:], sh[:], sin_t[:])
                for par in range(2):
                    o0 = par * DH
                    nc.vector.tensor_mul(
                        qz[o0:o0 + DH, mb, par, :],
                        qcT[o0:o0 + DH, mb, :],
                        cos_t[o0:o0 + DH, :],
                    )
                    nc.vector.tensor_add(
                        qz[o0:o0 + DH, mb, par, :],
                        qz[o0:o0 + DH, mb, par, :],
                        sh[o0:o0 + DH, :],
                    )

            proj(wk_t, kcT, 0)
            rope_k(0, nc.vector, rtmp)
            proj(wq_t, qcT, 0)
            rope_q(0, rtmp)
            proj(wk_t, kcT, 1)
            proj(wq_t, qcT, 1)
            proj(wv_t, vT, 0)
            proj(wv_t, vT, 1)
            rope_fns = (rope_k, rope_q)

            # V natural layout (t on partitions) + ones column per head
            nc.vector.memset(vn[:, :, :, 64:65], 1.0)
            for mb in range(MB):
                for sb_i in range(NSB):
                    ps = tp_ps.tile([P, P], bf16, tag="tp")
                    nc.tensor.transpose(
                        ps[:], vT[:, mb, sb_i * P:(sb_i + 1) * P], ident[:]
                    )
                    nc.vector.tensor_copy(
                        vn[:, sb_i, 2 * mb:2 * mb + 2, 0:64],
                        ps[:].rearrange("p (a b) -> p a b", a=2),
                    )

        # ---- Phase B: attention, two heads interleaved, q in halves ----
        # PSUM budget: 2 acc tiles (65, 1024) = 2 banks each + 2 sc bufs
        # (128, 1024) = 2 banks each -> 8 banks. Interleaving a head pair
        # keeps TensorE dense enough that HAM stays at full clock while
        # ACT runs the exps.
        with ExitStack() as bctx:
            sc_ps = bctx.enter_context(
                tc.tile_pool(name="sc_ps", bufs=2, space="PSUM")
            )
            ac_ps = bctx.enter_context(
                tc.tile_pool(name="ac_ps", bufs=1, space="PSUM")
            )
            ptp = bctx.enter_context(tc.tile_pool(name="ptp", bufs=4))
            dn = bctx.enter_context(tc.tile_pool(name="dn", bufs=2))

            # mb=1 RoPE runs here so it doesn't gate attention start
            # (the phase-B pools only open once ALL phase-A work is
            # done); its outputs are first needed mid-attention by the
            # second head pair.
            rope_fns[0](1, nc.gpsimd, dn)
            rope_fns[1](1, dn)

            deferred_norms = []
            for hp in range(2):
                heads = (2 * hp, 2 * hp + 1)
                for pss in range(2):
                    q0 = pss * 1024
                    accs = {
                        h: ac_ps.tile([65, 1024], f32, tag=f"acc{h % 2}",
                                      name=f"acc_{h}_{pss}")
                        for h in heads
                    }
                    def issue_pv(h, ti, pt, lo, hi):
                        # one PV piece per PSUM bank; bank bk (global)
                        # is complete at ti == 4*bk+3
                        p0 = lo
                        while p0 < hi:
                            bk = p0 // 512
                            p1 = min(hi, (bk + 1) * 512)
                            nc.tensor.matmul(
                                accs[h][:, p0 - q0:p1 - q0],
                                lhsT=vn[:, ti, h, :],
                                rhs=pt[:, p0 - q0:p1 - q0],
                                start=(ti == 0),
                                stop=(ti == 4 * bk + 3),
                            )
                            p0 = p1

                    # software pipeline: PV consumes the PREVIOUS
                    # iteration's exp output, so TensorE never waits on
                    # ScalarE inside an iteration (keeps the PE dense ->
                    # HAM stays at full clock; exp overlaps fully).
                    pending = []
                    for ti in range(8 if pss == 0 else NSB):
                        if ti == 2 and deferred_norms:
                            # previous pass's normalize chains, emitted
                            # here so the PSUM-release semaphores (which
                            # gate this pass's PV accumulators) are not
                            # queued behind ~8us of reciprocal work
                            for st in deferred_norms:
                                epilogue_norm(*st)
                            deferred_norms = []
                        if pss == 1 and ti == 13:
                            # PSUM bank 2 (cols q0..q0+512) got its last
                            # PV at ti==11: normalize it mid-loop while
                            # ti 13..15 still stream
                            for h2 in heads:
                                epilogue_norm(*epilogue_copies(h2, q0, q0 + 512))
                        t0 = ti * P
                        lo = max(t0, q0)
                        hi = q0 + 1024
                        new = []
                        scs = {}
                        for h in heads:
                            scs[h] = sc_ps.tile([P, 1024], f32, tag="sc",
                                                name=f"sc_{h}_{ti}")
                        p0 = lo
                        while p0 < hi:
                            p1 = min(hi, (p0 // 512 + 1) * 512)
                            for h in heads:
                                mb = h // 2
                                nc.tensor.matmul(
                                    scs[h][:, p0 - q0:p1 - q0],
                                    lhsT=kT[:, mb, t0:t0 + P],
                                    rhs=qz[:, mb, h % 2, p0:p1],
                                )
                            p0 = p1
                        for h in heads:
                            mb, off = h // 2, (h % 2) * DH
                            sc = scs[h]
                            pt = ptp.tile([P, 1024], bf16, tag="pt")
                            nc.scalar.activation(
                                pt[:, lo - q0:hi - q0],
                                sc[:, lo - q0:hi - q0],
                                AF.Exp,
                                scale=ATTN_SCALE,
                            )
                            if t0 >= q0:
                                nc.vector.tensor_mul(
                                    pt[:, t0 - q0:t0 - q0 + P],
                                    pt[:, t0 - q0:t0 - q0 + P],
                                    msk_t[:],
                                )
                            new.append((h, ti, pt, lo, hi))
                        for args in pending:
                            issue_pv(*args)
                        pending = new
                    for args in pending:
                        issue_pv(*args)
                    def epilogue_copies(h, c0, c1):
                        # stage out^T + the denominator row out of PSUM
                        # (plain DVE copies; custom-DVE ops mishandle
                        # PSUM/partition-offset inputs on HW). These two
                        # copies are all that holds the accumulator
                        # banks.
                        w = c1 - c0
                        l0 = c0 - q0
                        acb = dn.tile([DH, w], f32, tag=f"acb{h % 2}",
                                      name=f"acb{h}_{c0}")
                        nc.vector.tensor_copy(acb[:], accs[h][0:DH, l0:l0 + w])
                        den0 = dn.tile([1, w], f32, tag=f"den0{h % 2}",
                                       name=f"den0{h}_{c0}")
                        nc.vector.tensor_copy(
                            den0[:], accs[h][64:65, l0:l0 + w]
                        )
                        return h, c0, c1, acb, den0

                    def epilogue_norm(h, c0, c1, acb, den0):
                        # 2-pass approximate reciprocal (~22 bits), a
                        # partition broadcast on the (idle) GpSimd
                        # engine (NOTE: partition_broadcast on HW always
                        # reads the tile's partition 0, so rden must be
                        # a base-0 tile), then one multiply.
                        mb, off = h // 2, (h % 2) * DH
                        w = c1 - c0
                        rden = dn.tile([1, w], f32, tag=f"rden{h % 2}",
                                       name=f"rden{h}_{c0}")
                        rscr = dn.tile([1, w], f32, tag=f"rscr{h % 2}",
                                       name=f"rscr{h}_{c0}")
                        nc.vector.reciprocal_approx_accurate(
                            rden[:], den0[:], rscr[:]
                        )
                        rdb = dn.tile([DH, w], f32, tag="rdb",
                                      name=f"rdb{h}_{c0}")
                        nc.gpsimd.partition_broadcast(rdb[:], rden[:])
                        if debug:
                            nc.sync.dma_start(dbg["dacc"][:, h, c0:c1], acb[:])
                            nc.sync.dma_start(
                                dbg["dden"][:, h, c0:c1], rden[:]
                            )
                        nc.vector.tensor_mul(
                            onrm[off:off + DH, mb, c0:c1], acb[:], rdb[:]
                        )

                    if pss == 1:
                        chunks = [(h, q0 + 512, q0 + 1024) for h in heads]
                    else:
                        chunks = [(h, q0, q0 + 1024) for h in heads]
                    # stage 1 now (the PSUM-freeing copies); the
                    # reciprocal chains are deferred into the next
                    # pass's loop so the release semaphores on the DVE
                    # queue fire right after the copies.
                    staged = [epilogue_copies(*c) for c in chunks]
                    if hp == 1 and pss == 1:
                        for st in staged:
                            epilogue_norm(*st)
                    else:
                        deferred_norms = staged

        if debug:
            nc.sync.dma_start(dbg["dxT"], xT[:])
            nc.sync.dma_start(dbg["dqcT"], qcT[:])
            for _mb in range(MB):
                for _par in range(2):
                    _o0 = _par * DH
                    nc.sync.dma_start(
                        dbg["dqT"][_o0:_o0 + DH, _mb, :],
                        qz[_o0:_o0 + DH, _mb, _par, :],
                    )
            nc.sync.dma_start(dbg["dkT"], kT[:])
            nc.sync.dma_start(dbg["dvn"], vn[:])
            nc.sync.dma_start(dbg["donrm"], onrm[:])

        # ---- Phase C: output projection ----
        with ExitStack() as cctx:
            y_ps = cctx.enter_context(
                tc.tile_pool(name="y_ps", bufs=2, space="PSUM")
            )
            yo = cctx.enter_context(tc.tile_pool(name="yo", bufs=3))
            for sb_i in range(NSB):
                yp = y_ps.tile([P, E], f32, tag="yp")
                for mb in range(MB):
                    for half in range(2):
                        nc.tensor.matmul(
                            yp[:, half * 512:(half + 1) * 512],
                            lhsT=onrm[:, mb, sb_i * P:(sb_i + 1) * P],
                            rhs=wo_t[:, mb, half * 512:(half + 1) * 512],
                            start=(mb == 0),
                            stop=(mb == MB - 1),
                        )
                ys = yo.tile([P, E], f32, tag="ys")
                for half in range(2):
                    sl = slice(half * 512, (half + 1) * 512)
                    if (sb_i + half) % 2 == 0:
                        nc.vector.tensor_copy(ys[:, sl], yp[:, sl])
                    else:
                        nc.scalar.copy(ys[:, sl], yp[:, sl])
                    eng = nc.sync if half == 0 else nc.scalar
                    eng.dma_start(y[sb_i * P:(sb_i + 1) * P, sl], ys[:, sl])

    nc.compile()
    return nc


def get_program():
    global _PROG
    if _PROG is None:
        _PROG = _build_program()
    return _PROG


def make_in_maps(x, W_q, W_k, W_v, W_o):
    perm = _perm64()
    idx_local = (np.arange(DC) // 64) * 64 + perm[np.arange(DC) % 64]
    ang, sgn = _cos_sin_tiles()
    cos_np = np.cos(ang).astype(BF16)
    sin_np = (sgn * np.sin(ang)).astype(BF16)
    # scores tile is (t, q): keep t <= q -> upper triangular incl. diagonal
    cmask_np = np.triu(np.ones((P, P))).astype(BF16)
    in_maps = []
    for c in range(NCORES):
        b, hg = c // 4, c % 4
        base = hg * DC
        in_maps.append(
            dict(
                xbt=np.ascontiguousarray(x[b].T.astype(BF16)),
                wq=np.ascontiguousarray(W_q[:, base + idx_local].astype(BF16)),
                wk=np.ascontiguousarray(W_k[:, base + idx_local].astype(BF16)),
                wv=np.ascontiguousarray(W_v[:, base:base + DC].astype(BF16)),
                wo=np.ascontiguousarray(W_o[base:base + DC, :].astype(BF16)),
                cosr=cos_np,
                sinr=sin_np,
                cmask=cmask_np,
            )
        )
    return in_maps


def kernel(x, W_q, W_k, W_v, W_o, _trace=False, _trace_cores=None):
    from concourse.bass_utils import run_bass_kernel_spmd

    x = np.asarray(x, dtype=np.float32)
    W_q = np.asarray(W_q, dtype=np.float32)
    W_k = np.asarray(W_k, dtype=np.float32)
    W_v = np.asarray(W_v, dtype=np.float32)
    W_o = np.asarray(W_o, dtype=np.float32)

    nc = get_program()
    in_maps = make_in_maps(x, W_q, W_k, W_v, W_o)
    res = run_bass_kernel_spmd(
        nc,
        in_maps,
        list(range(NCORES)),
        trace=_trace,
        trace_cores=_trace_cores,
    )
    y = np.zeros((B, S, E), np.float32)
    for c in range(NCORES):
        y[c // 4] += res.results[c]["y"]
    if _trace:
        return y, res
    return y
